# revision 15
# baseline (speedup 1.0000x reference)
"""Trainium2 Bass kernel for the attention-LSTM captioning RNN.

Problem (per full batch): x(64,128,512), A(64,1024,4,4), Wx(512,4096),
Wh(1024,4096), Wattn(1024,4096), b(4096) -> h-sequence (64,128,1024).

Strategy: data-parallel over N across 8 cores (8 samples/core, weights
replicated).  Per core:
  - precompute P[(n,l),g] = Af[n,:,l] @ Wattn  (PE, fp32) -> SBUF bf16
  - precompute xWx^T (gate-major) once (PE, f32r) -> SBUF bf16, indexed
    per step with a strided AP (no per-step DMA)
  - recurrence in transposed ("a^T") layout: gates live on 128 partitions
    (partition = gate-col % 128), batch (8) on the free dim.  Wh is the
    stationary operand (bf16, FWL), h^T the 8-wide moving operand.
    Attention is folded in as accumulating matmuls with a block-diagonal
    softmax-weight matrix ee (128x8) against stationary P.

Pipelining (the perf-critical part): the h-update is split into two
halves (h-chunks 0-3 / 4-7).  The loop body unrolls two timesteps (A/B
phases) with ping-ponged h/aT/score buffers, and the PE instruction
stream is ordered so the next step's Wh matmuls over the finished half
run while the other half's gate math is still on the Vector engine.
This removes the per-step PE idle gap that otherwise re-throttles the
PE clock (HAM) every step.  Softmax exp runs on the Scalar engine
(exp+tanh share one activation-table set; a pre-loop dummy activation
keeps the table load out of the loop).
Host numpy does all layout transposes (free: not timed on device).
"""

import math
import sys

sys.path.insert(0, "/root/shim")
sys.path.insert(0, "/opt/trn_rl_repo")

import numpy as np
import ml_dtypes

try:
    import antenv

    if "/root/shim/antenv" not in list(antenv.__path__):
        antenv.__path__.append("/root/shim/antenv")
except Exception:
    pass

import concourse.bass as bass
import concourse.bacc as bacc
import concourse.mybir as mybir
from concourse.tile import TileContext
from concourse.bass_utils import run_bass_kernel_spmd

FP32 = mybir.dt.float32
F32R = mybir.dt.float32r
BF16 = mybir.dt.bfloat16

# Problem constants (hardcoded per harness contract)
N, T, D, H = 64, 128, 512, 1024
NC = 8            # cores
NL = N // NC      # samples per core = 8
G = 4 * H         # 4096 gate columns
L = 16            # attention locations
HC = H // 128     # 8 h-chunks
GM = G // 128     # 32 gate-col chunks
DC = D // 128     # 4 d-chunks
INV_SQRT_H = 1.0 / math.sqrt(H)

# psum slot for gate-tile m (m = q*8 + c; slot = c*4 + q so that an
# h-chunk's four gate quarters are contiguous slots)
def slot_of(m):
    q, c = m // 8, m % 8
    return c * 4 + q


def build_nc(timesteps=T):
    nc = bacc.Bacc()

    # ---- DRAM I/O (host-prepped layouts) ----
    xT_d = nc.dram_tensor("xT", [128, DC, NL, timesteps], FP32, kind="ExternalInput")
    afT_d = nc.dram_tensor("afT", [128, HC, NL, L], FP32, kind="ExternalInput")
    wx_d = nc.dram_tensor("wx", [128, DC, G], FP32, kind="ExternalInput")
    wh_d = nc.dram_tensor("wh", [128, HC, G], BF16, kind="ExternalInput")
    wattn_d = nc.dram_tensor("wattn", [128, HC, G], FP32, kind="ExternalInput")
    b_d = nc.dram_tensor("bias", [128, GM], FP32, kind="ExternalInput")
    mask_d = nc.dram_tensor("mask", [128, NL], FP32, kind="ExternalInput")
    bmask_d = nc.dram_tensor("bmask", [128, 128], FP32, kind="ExternalInput")
    out_d = nc.dram_tensor("hsT", [timesteps, 128, HC, NL], BF16, kind="ExternalOutput")

    half_ms = [[q * 8 + c for c in range(4) for q in range(4)],
               [q * 8 + c for c in range(4, 8) for q in range(4)]]

    with TileContext(nc) as tc:
        # ---------- persistent SBUF ----------
        with tc.tile_pool(name="persist", bufs=1) as pp:
            afTb = pp.tile([128, HC, NL, L], BF16)     # Af^T bf16, (hc,n,l) free
            p_sb = pp.tile([128, G], BF16)             # P[(n,l), g]
            wh_sb = pp.tile([128, HC, G], BF16)        # Wh tiles
            xwxt = pp.tile([128, GM, NL, timesteps], BF16)  # xWx^T (+bias), slot-ordered
            bias_sb = pp.tile([128, GM], FP32)
            mask_sb = pp.tile([128, NL], FP32)
            bmask_sb = pp.tile([128, 128], FP32)       # 16-block partition mask
            hT = [pp.tile([128, HC, NL], BF16, name=f"hT{i}") for i in range(2)]
            cT = pp.tile([128, HC, NL], FP32)
            chT = pp.tile([128, HC, NL], FP32)         # c/2
            mask2_sb = pp.tile([128, 2, NL], FP32)     # 4*mask, twice
            scratch1 = pp.tile([128, 1], FP32)

            nc.sync.dma_start(bias_sb[:], b_d[:])
            nc.sync.dma_start(mask_sb[:], mask_d[:])
            nc.sync.dma_start(bmask_sb[:], bmask_d[:])

            # ---------- P = Af^T @ Wattn  (fp32, one-time) ----------
            with (
                tc.tile_pool(name="wattn", bufs=1) as wap,
                tc.tile_pool(name="wsl", bufs=2) as wslp,
                tc.tile_pool(name="ppsum", bufs=1, space="PSUM") as ppp,
            ):
                afT = wap.tile([128, HC, NL, L], FP32)
                nc.sync.dma_start(afT[:], afT_d[:])
                # afTb scaled 1/256: exp argument becomes 4*s directly
                nc.vector.tensor_scalar_mul(afTb[:], afT[:], 1.0 / 256.0)

                # h0 = mean over l of Af  (h2 = 2*h convention)
                nc.vector.tensor_reduce(
                    cT[:], afT[:], axis=mybir.AxisListType.X,
                    op=mybir.AluOpType.add,
                )
                nc.vector.tensor_scalar_mul(cT[:], cT[:], 1.0 / L)
                nc.vector.tensor_scalar_mul(hT[1][:], cT[:], 2.0)
                nc.vector.tensor_scalar_mul(chT[:], cT[:], 0.5)
                nc.vector.tensor_scalar_mul(mask2_sb[:, 0, :], mask_sb[:], 4.0)
                nc.vector.tensor_scalar_mul(mask2_sb[:, 1, :], mask_sb[:], 4.0)

                pps = [ppp.tile([128, 1024], FP32, tag=f"pps{gc}", name=f"pps{gc}") for gc in range(4)]
                for hc in range(HC):
                    wsl = wslp.tile([128, G], FP32, tag="wsl")
                    nc.sync.dma_start(wsl[:], wattn_d[:, hc, :])
                    for gc in range(4):
                        for hf in range(2):
                            nc.tensor.matmul(
                                pps[gc][:, hf * 512:(hf + 1) * 512],
                                afT[:, hc, :, :],
                                wsl[
                                    :,
                                    gc * 1024 + hf * 512:gc * 1024 + (hf + 1) * 512,
                                ],
                                start=(hc == 0),
                                stop=(hc == HC - 1),
                            )
                for gc in range(4):
                    nc.vector.tensor_copy(
                        p_sb[:, gc * 1024:(gc + 1) * 1024], pps[gc][:]
                    )

            # ---------- xWx^T into SBUF bf16 (f32r, one-time) ----------
            with (
                tc.tile_pool(name="xwx", bufs=1) as xp,
                tc.tile_pool(name="xwxs", bufs=2) as xsp,
                tc.tile_pool(name="xwpsum", bufs=1, space="PSUM") as xpp,
            ):
                xT_r = xp.tile([128, DC, NL, timesteps], F32R)
                for dc in range(DC):
                    st2 = xsp.tile([128, NL * timesteps], FP32, tag="stage2")
                    nc.sync.dma_start(
                        st2[:], xT_d[:, dc, :, :].rearrange("p n t -> p (n t)")
                    )
                    nc.vector.tensor_copy(
                        xT_r[:, dc, :, :].rearrange("p n t -> p (n t)"), st2[:]
                    )
                ncols = NL * timesteps  # 1024
                col_chunks = [(s, min(s + 512, ncols)) for s in range(0, ncols, 512)]
                for mg in range(GM // 4):  # groups of 4 gate-chunks
                    xwg = [
                        xpp.tile([128, ncols], FP32, tag=f"xw{i}", name=f"xw{i}")
                        for i in range(4)
                    ]
                    for dc in range(DC):
                        st = xsp.tile([128, 512], FP32, tag="stage")
                        nc.sync.dma_start(
                            st[:], wx_d[:, dc, mg * 512:(mg + 1) * 512]
                        )
                        wxr = xsp.tile([128, 512], F32R, tag="wxr")
                        nc.vector.tensor_copy(wxr[:], st[:])
                        for i in range(4):
                            for (lo, hi) in col_chunks:
                                nc.tensor.matmul(
                                    xwg[i][:, lo:hi],
                                    wxr[:, i * 128:(i + 1) * 128],
                                    xT_r[:, dc, :, :].rearrange(
                                        "p n t -> p (n t)"
                                    )[:, lo:hi],
                                    start=(dc == 0),
                                    stop=(dc == DC - 1),
                                )
                    for i in range(4):
                        m = mg * 4 + i
                        nc.vector.tensor_scalar_add(
                            xwxt[:, slot_of(m), :, :].rearrange("p n t -> p (n t)"),
                            xwg[i][:],
                            bias_sb[:, m:m + 1],
                        )

            # Wh load (bf16, direct)
            nc.sync.dma_start(wh_sb[:], wh_d[:])

            # warm the exp/tanh activation-table set so the in-loop
            # activations never trigger an ACT_TABLE_LOAD
            nc.scalar.activation(
                scratch1[:], bias_sb[:, 0:1], mybir.ActivationFunctionType.Exp
            )
            nc.scalar.activation(
                scratch1[:], scratch1[:], mybir.ActivationFunctionType.Tanh
            )

            # ---------- recurrence ----------
            # PSUM rule: the first matmul of an accumulation group
            # (start=True) clears has_written for the WHOLE bank, so each
            # group's matmuls must be contiguous on the PE stream and a
            # group must be closed before any other group in the same
            # bank starts.  Hence: per-slot contiguous kc0..3 chains into
            # aTlo, per-slot kc4..7 chains into aThi, single-matmul
            # attention contributions into uT, and split score tiles.
            with (
                tc.tile_pool(name="step", bufs=2) as sp,
                tc.tile_pool(name="fchain", bufs=1) as fsp,
                tc.tile_pool(name="gpsum", bufs=1, space="PSUM") as gp,
            ):
                ab = [gp.tile([128, 2, GM, NL], FP32, name=f"ab{i}") for i in range(2)]
                aTlo = [t[:, 0] for t in ab]
                aThi = [t[:, 1] for t in ab]
                uT = [gp.tile([128, GM, NL], FP32, name=f"uT{i}") for i in range(2)]
                scz = [gp.tile([128, 3, NL], FP32, name=f"scz{i}") for i in range(2)]
                sc1 = [t[:, 0, :] for t in scz]
                sc2 = [t[:, 1, :] for t in scz]
                zsp = [t[:, 2, 0:1] for t in scz]

                def sc_mms(dst, hsrc, kcs):
                    # scores matmuls: dst[(n,l), n'] += afTb[kc]^T @ h[kc]
                    for kc in kcs:
                        nc.tensor.matmul(
                            dst,
                            afTb[:, kc, :, :].rearrange("p n l -> p (n l)"),
                            hsrc[:, kc, :],
                            start=(kc == kcs[0]),
                            stop=(kc == kcs[-1]),
                        )

                def wh_mms(dst, hsrc, ms, kcs):
                    for m in ms:
                        s = slot_of(m)
                        for kc in kcs:
                            nc.tensor.matmul(
                                dst[:, s, :],
                                wh_sb[:, kc, m * 128:(m + 1) * 128],
                                hsrc[:, kc, :],
                                start=(kc == kcs[0]),
                                stop=(kc == kcs[-1]),
                            )

                def e_mms(ph, ee, ms):
                    for m in ms:
                        nc.tensor.matmul(
                            uT[ph][:, slot_of(m), :],
                            p_sb[:, m * 128:(m + 1) * 128],
                            ee[:],
                            start=True,
                            stop=True,
                        )

                def gate_pre(ph, half, t_idx):
                    # gs = aTlo + aThi + xwxt[t]  (uT added later, off the
                    # critical path of the attention matmuls)
                    lo = half * 16
                    gs = fsp.tile([128, 16, NL], FP32, tag="gs")
                    nc.vector.tensor_tensor(
                        gs[:].rearrange("p m n -> p (m n)").unsqueeze(2),
                        aTlo[ph][:, lo:lo + 16, :].rearrange("p m n -> p (m n)").unsqueeze(2),
                        xwxt[:, lo:lo + 16, :, bass.ds(t_idx, 1)].rearrange(
                            "p m n t -> p (m n) t"
                        ),
                        mybir.AluOpType.add,
                    )
                    nc.vector.tensor_tensor(
                        gs[:].rearrange("p m n -> p (m n)"),
                        gs[:].rearrange("p m n -> p (m n)"),
                        aThi[ph][:, lo:lo + 16, :].rearrange("p m n -> p (m n)"),
                        mybir.AluOpType.add,
                    )
                    return gs

                def gate_fin(ph, half, gs, hw):
                    lo = half * 16
                    gl = fsp.tile([128, 16, NL], FP32, tag="gl")
                    nc.vector.tensor_tensor(
                        gl[:].rearrange("p m n -> p (m n)"),
                        gs[:].rearrange("p m n -> p (m n)"),
                        uT[ph][:, lo:lo + 16, :].rearrange("p m n -> p (m n)"),
                        mybir.AluOpType.add,
                    )
                    nc.scalar.activation(
                        gl[:].rearrange("p m n -> p (m n)"),
                        gl[:].rearrange("p m n -> p (m n)"),
                        mybir.ActivationFunctionType.Tanh,
                    )
                    gv = gl[:].rearrange("p (c q) n -> p c q n", q=4)
                    c_half = cT[:, half * 4:half * 4 + 4, :]
                    ch_half = chT[:, half * 4:half * 4 + 4, :]
                    # tail on GPSIMD (all-SBUF): frees the DVE queue
                    # u2 = (ti+1)*tg = 2*i*g ; t1 = (tf+1)*(c/2) = f*c
                    u2 = sp.tile([128, 4, NL], FP32, tag=f"u2{half}")
                    nc.vector.scalar_tensor_tensor(
                        u2[:], gv[:, :, 0, :], 1.0, gv[:, :, 3, :],
                        mybir.AluOpType.add, mybir.AluOpType.mult,
                    )
                    t1 = sp.tile([128, 4, NL], FP32, tag=f"t1{half}")
                    nc.vector.scalar_tensor_tensor(
                        t1[:], gv[:, :, 1, :], 1.0, ch_half,
                        mybir.AluOpType.add, mybir.AluOpType.mult,
                    )
                    # c' = u2/2 + t1
                    nc.vector.scalar_tensor_tensor(
                        c_half, u2[:], 0.5, t1[:],
                        mybir.AluOpType.mult, mybir.AluOpType.add,
                    )
                    tct = sp.tile([128, 4, NL], FP32, tag=f"tct{half}")
                    nc.scalar.activation(
                        tct[:], c_half, mybir.ActivationFunctionType.Tanh
                    )
                    # h2 = (to+1)*tanh(c)  (bf16 directly)
                    nc.vector.scalar_tensor_tensor(
                        hw[:, half * 4:half * 4 + 4, :],
                        gv[:, :, 2, :], 1.0, tct[:],
                        mybir.AluOpType.add, mybir.AluOpType.mult,
                    )
                    nc.vector.tensor_scalar_mul(ch_half, c_half, 0.5)

                def phase(ph, t_idx):
                    hr, hw = hT[1 - ph], hT[ph]
                    # S1a + S1b: all lo chains -- need only hr[0..3], so they
                    # cover the previous phase's half1 gate chain entirely
                    wh_mms(aTlo[ph], hr, half_ms[0], range(0, 4))
                    wh_mms(aTlo[ph], hr, half_ms[1], range(0, 4))
                    # S9: this step's score tail, kc 4..7 (needs hr[4..7])
                    sc_mms(sc2[ph], hr, range(4, 8))
                    # S3: hi chains, half0
                    wh_mms(aThi[ph], hr, half_ms[0][:8], range(4, 8))
                    # softmax (DVE + Scalar), runs under S1a..S3.
                    # s_tot = 4*sum_n mask*(sc1+sc2); e_col = exp(s_tot)
                    junk = sp.tile([128, 2, NL], FP32, tag="junk")
                    s_tot = sp.tile([128, 1], FP32, tag="s_tot")
                    nc.vector.scalar_tensor_tensor(
                        junk[:], scz[ph][:, 0:2, :], 1.0, mask2_sb[:],
                        mybir.AluOpType.mult, mybir.AluOpType.mult,
                        accum_out=s_tot[:],
                    )
                    e_col = sp.tile([128, 1], FP32, tag="e_col")
                    nc.scalar.activation(
                        e_col[:], s_tot[:], mybir.ActivationFunctionType.Exp,
                    )
                    # S4: Z via block-diag partition sums
                    nc.tensor.matmul(
                        zsp[ph], bmask_sb[:], e_col[:], start=True, stop=True
                    )
                    # S3 part 2: hi chains, rest of half0
                    wh_mms(aThi[ph], hr, half_ms[0][8:], range(4, 8))
                    # gs(half0) partials fill the DVE while zsp is in flight
                    gs0 = gate_pre(ph, 0, t_idx)
                    rz = sp.tile([128, 1], FP32, tag="rz")
                    nc.vector.reciprocal(rz[:], zsp[ph])
                    ee = sp.tile([128, NL], BF16, tag="ee")
                    nc.vector.tensor_scalar(
                        ee[:], mask_sb[:], e_col[:, 0:1], rz[:, 0:1],
                        mybir.AluOpType.mult, mybir.AluOpType.mult,
                    )
                    # S5: attention matmuls half0 (early: starts gate chain)
                    e_mms(ph, ee, half_ms[0])
                    # V: finish gates half0 -> hw[0:4]
                    gate_fin(ph, 0, gs0, hw)
                    # S6: hi chains, half1
                    wh_mms(aThi[ph], hr, half_ms[1], range(4, 8))
                    # S7: attention matmuls half1
                    e_mms(ph, ee, half_ms[1])
                    gs1 = gate_pre(ph, 1, t_idx)
                    # S8: next step's scores kc 0..3 (consume hw half0)
                    sc_mms(sc1[1 - ph], hw, range(0, 4))
                    # V: finish gates half1 -> hw[4:8]
                    gate_fin(ph, 1, gs1, hw)
                    # output
                    nc.sync.dma_start(
                        out_d[bass.ds(t_idx, 1), :, :, :].rearrange(
                            "t p c n -> p (t c) n"
                        ),
                        hw[:],
                    )

                # prologue: scores kc 0..3 for step 0
                sc_mms(sc1[0], hT[1], range(0, 4))

                with tc.For_i(0, timesteps, 16, staggered_reset=True) as ti:
                    for k in range(16):
                        phase(k % 2, ti + k)

    nc.finalize()
    return nc


def prep_inputs(x, A, Wx, Wh, Wattn, b):
    """Host-side reshapes to device layouts; returns per-core input maps."""
    x = np.asarray(x, dtype=np.float32)
    A = np.asarray(A, dtype=np.float32)
    Wx = np.asarray(Wx, dtype=np.float32)
    Wh = np.asarray(Wh, dtype=np.float32)
    Wattn = np.asarray(Wattn, dtype=np.float32)
    b = np.asarray(b, dtype=np.float32)
    timesteps = x.shape[1]

    # weight layouts [p, kc, g] with k = kc*128 + p
    # per-gate-column scaling: i/f/o columns carry a 0.5 (tanh half-angle
    # trick), g columns stay full-scale; Wh gets an extra 0.5 (h2 = 2h).
    gsc = np.ones((G,), np.float32) * 0.5
    gsc[3 * H:] = 1.0
    wx_h = np.ascontiguousarray((gsc * Wx).reshape(DC, 128, G).transpose(1, 0, 2))
    wh_h = np.ascontiguousarray(
        ((0.5 * gsc) * Wh).reshape(HC, 128, G).transpose(1, 0, 2).astype(
            ml_dtypes.bfloat16
        )
    )
    wattn_h = np.ascontiguousarray(
        (gsc * Wattn).reshape(HC, 128, G).transpose(1, 0, 2)
    )
    b_h = np.ascontiguousarray((gsc * b).reshape(GM, 128).T)  # [p, m]
    mask_h = np.zeros((128, NL), dtype=np.float32)
    for p in range(128):
        mask_h[p, p // L] = 1.0
    bmask_h = (np.arange(128)[:, None] // L == np.arange(128)[None, :] // L).astype(
        np.float32
    )

    in_maps = []
    for c in range(NC):
        xs = x[c * NL:(c + 1) * NL]          # (8, T, 512)
        As = A[c * NL:(c + 1) * NL].reshape(NL, H, L)  # (8, 1024, 16)
        # xT [p, dc, n, t] = x[n, t, dc*128+p]
        xT_h = np.ascontiguousarray(
            xs.reshape(NL, timesteps, DC, 128).transpose(3, 2, 0, 1)
        )
        # afT [p, hc, n, l] = Af[n, hc*128+p, l]
        afT_h = np.ascontiguousarray(
            As.reshape(NL, HC, 128, L).transpose(2, 1, 0, 3)
        )
        in_maps.append(
            {
                "xT": xT_h,
                "afT": afT_h,
                "wx": wx_h,
                "wh": wh_h,
                "wattn": wattn_h,
                "bias": b_h,
                "mask": mask_h,
                "bmask": bmask_h,
            }
        )
    return in_maps


_NC_CACHE = {}


def kernel(x, A, Wx, Wh, Wattn, b, trace=False):
    timesteps = x.shape[1]
    key = timesteps
    if key not in _NC_CACHE:
        _NC_CACHE[key] = build_nc(timesteps)
    nc = _NC_CACHE[key]
    in_maps = prep_inputs(x, A, Wx, Wh, Wattn, b)
    res = run_bass_kernel_spmd(nc, in_maps, list(range(NC)), trace=trace)
    outs = []
    for c in range(NC):
        hsT = res.results[c]["hsT"]  # (T, 128, HC, NL)
        # out[n, t, hc*128+p] = hsT[t, p, hc, n]
        outs.append(0.5 * hsT.astype(np.float32).transpose(3, 0, 2, 1).reshape(NL, timesteps, H))
    full = np.concatenate(outs, axis=0).astype(np.float32)
    kernel.last_result = res
    return full


# revision 16
# speedup vs baseline: 1.0183x; 1.0183x over previous
"""Trainium2 Bass kernel for the attention-LSTM captioning RNN.

Problem (per full batch): x(64,128,512), A(64,1024,4,4), Wx(512,4096),
Wh(1024,4096), Wattn(1024,4096), b(4096) -> h-sequence (64,128,1024).

Strategy: data-parallel over N across 8 cores (8 samples/core, weights
replicated).  Per core:
  - precompute P[(n,l),g] = Af[n,:,l] @ Wattn  (PE, fp32) -> SBUF bf16
  - precompute xWx^T (gate-major) once (PE, f32r) -> SBUF bf16, indexed
    per step with a strided AP (no per-step DMA)
  - recurrence in transposed ("a^T") layout: gates live on 128 partitions
    (partition = gate-col % 128), batch (8) on the free dim.  Wh is the
    stationary operand (bf16, FWL), h^T the 8-wide moving operand.
    Attention is folded in as accumulating matmuls with a block-diagonal
    softmax-weight matrix ee (128x8) against stationary P.

Pipelining (the perf-critical part): the h-update is split into two
halves (h-chunks 0-3 / 4-7).  The loop body unrolls two timesteps (A/B
phases) with ping-ponged h/aT/score buffers, and the PE instruction
stream is ordered so the next step's Wh matmuls over the finished half
run while the other half's gate math is still on the Vector engine.
This removes the per-step PE idle gap that otherwise re-throttles the
PE clock (HAM) every step.  Softmax exp runs on the Scalar engine
(exp+tanh share one activation-table set; a pre-loop dummy activation
keeps the table load out of the loop).
Host numpy does all layout transposes (free: not timed on device).
"""

import math
import sys

sys.path.insert(0, "/root/shim")
sys.path.insert(0, "/opt/trn_rl_repo")

import numpy as np
import ml_dtypes

try:
    import antenv

    if "/root/shim/antenv" not in list(antenv.__path__):
        antenv.__path__.append("/root/shim/antenv")
except Exception:
    pass

import concourse.bass as bass
import concourse.bacc as bacc
import concourse.mybir as mybir
from concourse.tile import TileContext
from concourse.bass_utils import run_bass_kernel_spmd

FP32 = mybir.dt.float32
F32R = mybir.dt.float32r
BF16 = mybir.dt.bfloat16

# Problem constants (hardcoded per harness contract)
N, T, D, H = 64, 128, 512, 1024
NC = 8            # cores
NL = N // NC      # samples per core = 8
G = 4 * H         # 4096 gate columns
L = 16            # attention locations
HC = H // 128     # 8 h-chunks
GM = G // 128     # 32 gate-col chunks
DC = D // 128     # 4 d-chunks
INV_SQRT_H = 1.0 / math.sqrt(H)

# psum slot for gate-tile m (m = q*8 + c; slot = c*4 + q so that an
# h-chunk's four gate quarters are contiguous slots)
def slot_of(m):
    q, c = m // 8, m % 8
    return c * 4 + q


def build_nc(timesteps=T):
    nc = bacc.Bacc()

    # ---- DRAM I/O (host-prepped layouts) ----
    xT_d = nc.dram_tensor("xT", [128, DC, NL, timesteps], FP32, kind="ExternalInput")
    afT_d = nc.dram_tensor("afT", [128, HC, NL, L], FP32, kind="ExternalInput")
    wx_d = nc.dram_tensor("wx", [128, DC, G], FP32, kind="ExternalInput")
    wh_d = nc.dram_tensor("wh", [128, HC, G], BF16, kind="ExternalInput")
    wattn_d = nc.dram_tensor("wattn", [128, HC, G], FP32, kind="ExternalInput")
    b_d = nc.dram_tensor("bias", [128, GM], FP32, kind="ExternalInput")
    mask_d = nc.dram_tensor("mask", [128, NL], FP32, kind="ExternalInput")
    bmask_d = nc.dram_tensor("bmask", [128, 128], FP32, kind="ExternalInput")
    out_d = nc.dram_tensor("hsT", [timesteps, 128, HC, NL], BF16, kind="ExternalOutput")

    half_ms = [[q * 8 + c for c in range(4) for q in range(4)],
               [q * 8 + c for c in range(4, 8) for q in range(4)]]

    with TileContext(nc) as tc:
        # ---------- persistent SBUF ----------
        with tc.tile_pool(name="persist", bufs=1) as pp:
            afTb = pp.tile([128, HC, NL, L], BF16)     # Af^T bf16, (hc,n,l) free
            p_sb = pp.tile([128, G], BF16)             # P[(n,l), g]
            wh_sb = pp.tile([128, HC, G], BF16)        # Wh tiles
            xwxt = pp.tile([128, GM, NL, timesteps], BF16)  # xWx^T (+bias), slot-ordered
            bias_sb = pp.tile([128, GM], FP32)
            mask_sb = pp.tile([128, NL], FP32)
            bmask_sb = pp.tile([128, 128], FP32)       # 16-block partition mask
            hT = [pp.tile([128, HC, NL], BF16, name=f"hT{i}") for i in range(2)]
            cT = pp.tile([128, HC, NL], FP32)
            chT = pp.tile([128, HC, NL], FP32)         # c/2
            mask2_sb = pp.tile([128, 2, NL], FP32)     # 4*mask, twice
            scratch1 = pp.tile([128, 1], FP32)

            nc.sync.dma_start(bias_sb[:], b_d[:])
            nc.sync.dma_start(mask_sb[:], mask_d[:])
            nc.sync.dma_start(bmask_sb[:], bmask_d[:])

            # ---------- P = Af^T @ Wattn  (fp32, one-time) ----------
            with (
                tc.tile_pool(name="wattn", bufs=1) as wap,
                tc.tile_pool(name="wsl", bufs=2) as wslp,
                tc.tile_pool(name="ppsum", bufs=1, space="PSUM") as ppp,
            ):
                afT = wap.tile([128, HC, NL, L], FP32)
                nc.sync.dma_start(afT[:], afT_d[:])
                # afTb scaled 1/256: exp argument becomes 4*s directly
                nc.vector.tensor_scalar_mul(afTb[:], afT[:], 1.0 / 256.0)

                # h0 = mean over l of Af  (h2 = 2*h convention)
                nc.vector.tensor_reduce(
                    cT[:], afT[:], axis=mybir.AxisListType.X,
                    op=mybir.AluOpType.add,
                )
                nc.vector.tensor_scalar_mul(cT[:], cT[:], 1.0 / L)
                nc.vector.tensor_scalar_mul(hT[1][:], cT[:], 2.0)
                nc.vector.tensor_scalar_mul(chT[:], cT[:], 0.5)
                nc.vector.tensor_scalar_mul(mask2_sb[:, 0, :], mask_sb[:], 4.0)
                nc.vector.tensor_scalar_mul(mask2_sb[:, 1, :], mask_sb[:], 4.0)

                pps = [ppp.tile([128, 1024], FP32, tag=f"pps{gc}", name=f"pps{gc}") for gc in range(4)]
                for hc in range(HC):
                    wsl = wslp.tile([128, G], FP32, tag="wsl")
                    nc.sync.dma_start(wsl[:], wattn_d[:, hc, :])
                    for gc in range(4):
                        for hf in range(2):
                            nc.tensor.matmul(
                                pps[gc][:, hf * 512:(hf + 1) * 512],
                                afT[:, hc, :, :],
                                wsl[
                                    :,
                                    gc * 1024 + hf * 512:gc * 1024 + (hf + 1) * 512,
                                ],
                                start=(hc == 0),
                                stop=(hc == HC - 1),
                            )
                for gc in range(4):
                    nc.vector.tensor_copy(
                        p_sb[:, gc * 1024:(gc + 1) * 1024], pps[gc][:]
                    )

            # ---------- xWx^T into SBUF bf16 (f32r, one-time) ----------
            with (
                tc.tile_pool(name="xwx", bufs=1) as xp,
                tc.tile_pool(name="xwxs", bufs=2) as xsp,
                tc.tile_pool(name="xwpsum", bufs=1, space="PSUM") as xpp,
            ):
                xT_r = xp.tile([128, DC, NL, timesteps], F32R)
                for dc in range(DC):
                    st2 = xsp.tile([128, NL * timesteps], FP32, tag="stage2")
                    nc.sync.dma_start(
                        st2[:], xT_d[:, dc, :, :].rearrange("p n t -> p (n t)")
                    )
                    nc.vector.tensor_copy(
                        xT_r[:, dc, :, :].rearrange("p n t -> p (n t)"), st2[:]
                    )
                ncols = NL * timesteps  # 1024
                col_chunks = [(s, min(s + 512, ncols)) for s in range(0, ncols, 512)]
                for mg in range(GM // 4):  # groups of 4 gate-chunks
                    xwg = [
                        xpp.tile([128, ncols], FP32, tag=f"xw{i}", name=f"xw{i}")
                        for i in range(4)
                    ]
                    for dc in range(DC):
                        st = xsp.tile([128, 512], FP32, tag="stage")
                        nc.sync.dma_start(
                            st[:], wx_d[:, dc, mg * 512:(mg + 1) * 512]
                        )
                        wxr = xsp.tile([128, 512], F32R, tag="wxr")
                        nc.vector.tensor_copy(wxr[:], st[:])
                        for i in range(4):
                            for (lo, hi) in col_chunks:
                                nc.tensor.matmul(
                                    xwg[i][:, lo:hi],
                                    wxr[:, i * 128:(i + 1) * 128],
                                    xT_r[:, dc, :, :].rearrange(
                                        "p n t -> p (n t)"
                                    )[:, lo:hi],
                                    start=(dc == 0),
                                    stop=(dc == DC - 1),
                                )
                    for i in range(4):
                        m = mg * 4 + i
                        nc.vector.tensor_scalar_add(
                            xwxt[:, slot_of(m), :, :].rearrange("p n t -> p (n t)"),
                            xwg[i][:],
                            bias_sb[:, m:m + 1],
                        )

            # Wh load (bf16, direct)
            nc.sync.dma_start(wh_sb[:], wh_d[:])

            # warm the exp/tanh activation-table set so the in-loop
            # activations never trigger an ACT_TABLE_LOAD
            nc.scalar.activation(
                scratch1[:], bias_sb[:, 0:1], mybir.ActivationFunctionType.Exp
            )
            nc.scalar.activation(
                scratch1[:], scratch1[:], mybir.ActivationFunctionType.Tanh
            )

            # ---------- recurrence ----------
            # PSUM rule: the first matmul of an accumulation group
            # (start=True) clears has_written for the WHOLE bank, so each
            # group's matmuls must be contiguous on the PE stream and a
            # group must be closed before any other group in the same
            # bank starts.  Hence: per-slot contiguous kc0..3 chains into
            # aTlo, per-slot kc4..7 chains into aThi, single-matmul
            # attention contributions into uT, and split score tiles.
            with (
                tc.tile_pool(name="step", bufs=2) as sp,
                tc.tile_pool(name="fchain", bufs=1) as fsp,
                tc.tile_pool(name="gpsum", bufs=1, space="PSUM") as gp,
            ):
                ab = [gp.tile([128, 2, GM, NL], FP32, name=f"ab{i}") for i in range(2)]
                aTlo = [t[:, 0] for t in ab]
                aThi = [t[:, 1] for t in ab]
                uT = [gp.tile([128, GM, NL], FP32, name=f"uT{i}") for i in range(2)]
                scz = [gp.tile([128, 3, NL], FP32, name=f"scz{i}") for i in range(2)]
                sc1 = [t[:, 0, :] for t in scz]
                sc2 = [t[:, 1, :] for t in scz]
                zsp = [t[:, 2, 0:1] for t in scz]

                def sc_mms(dst, hsrc, kcs):
                    # scores matmuls: dst[(n,l), n'] += afTb[kc]^T @ h[kc]
                    for kc in kcs:
                        nc.tensor.matmul(
                            dst,
                            afTb[:, kc, :, :].rearrange("p n l -> p (n l)"),
                            hsrc[:, kc, :],
                            start=(kc == kcs[0]),
                            stop=(kc == kcs[-1]),
                        )

                def wh_mms(dst, hsrc, ms, kcs):
                    for m in ms:
                        s = slot_of(m)
                        for kc in kcs:
                            nc.tensor.matmul(
                                dst[:, s, :],
                                wh_sb[:, kc, m * 128:(m + 1) * 128],
                                hsrc[:, kc, :],
                                start=(kc == kcs[0]),
                                stop=(kc == kcs[-1]),
                            )

                def e_mms(ph, ee, ms):
                    for m in ms:
                        nc.tensor.matmul(
                            uT[ph][:, slot_of(m), :],
                            p_sb[:, m * 128:(m + 1) * 128],
                            ee[:],
                            start=True,
                            stop=True,
                        )

                def gate_pre(ph, half, t_idx):
                    # gs = aTlo + aThi + xwxt[t]  (uT added later, off the
                    # critical path of the attention matmuls)
                    lo = half * 16
                    gs = fsp.tile([128, 16, NL], FP32, tag="gs")
                    nc.vector.tensor_tensor(
                        gs[:].rearrange("p m n -> p (m n)").unsqueeze(2),
                        aTlo[ph][:, lo:lo + 16, :].rearrange("p m n -> p (m n)").unsqueeze(2),
                        xwxt[:, lo:lo + 16, :, bass.ds(t_idx, 1)].rearrange(
                            "p m n t -> p (m n) t"
                        ),
                        mybir.AluOpType.add,
                    )
                    nc.vector.tensor_tensor(
                        gs[:].rearrange("p m n -> p (m n)"),
                        gs[:].rearrange("p m n -> p (m n)"),
                        aThi[ph][:, lo:lo + 16, :].rearrange("p m n -> p (m n)"),
                        mybir.AluOpType.add,
                    )
                    return gs

                def gate_fin(ph, half, gs, hw):
                    lo = half * 16
                    gl = fsp.tile([128, 16, NL], FP32, tag="gl")
                    nc.vector.tensor_tensor(
                        gl[:].rearrange("p m n -> p (m n)"),
                        gs[:].rearrange("p m n -> p (m n)"),
                        uT[ph][:, lo:lo + 16, :].rearrange("p m n -> p (m n)"),
                        mybir.AluOpType.add,
                    )
                    nc.scalar.activation(
                        gl[:].rearrange("p m n -> p (m n)"),
                        gl[:].rearrange("p m n -> p (m n)"),
                        mybir.ActivationFunctionType.Tanh,
                    )
                    gv = gl[:].rearrange("p (c q) n -> p c q n", q=4)
                    c_half = cT[:, half * 4:half * 4 + 4, :]
                    ch_half = chT[:, half * 4:half * 4 + 4, :]
                    # tail on GPSIMD (all-SBUF): frees the DVE queue
                    # u2 = (ti+1)*tg = 2*i*g ; t1 = (tf+1)*(c/2) = f*c
                    u2 = sp.tile([128, 4, NL], FP32, tag=f"u2{half}")
                    nc.vector.scalar_tensor_tensor(
                        u2[:], gv[:, :, 0, :], 1.0, gv[:, :, 3, :],
                        mybir.AluOpType.add, mybir.AluOpType.mult,
                    )
                    t1 = sp.tile([128, 4, NL], FP32, tag=f"t1{half}")
                    nc.vector.scalar_tensor_tensor(
                        t1[:], gv[:, :, 1, :], 1.0, ch_half,
                        mybir.AluOpType.add, mybir.AluOpType.mult,
                    )
                    # c' = u2/2 + t1
                    nc.vector.scalar_tensor_tensor(
                        c_half, u2[:], 0.5, t1[:],
                        mybir.AluOpType.mult, mybir.AluOpType.add,
                    )
                    tct = sp.tile([128, 4, NL], FP32, tag=f"tct{half}")
                    nc.scalar.activation(
                        tct[:], c_half, mybir.ActivationFunctionType.Tanh
                    )
                    # h2 = (to+1)*tanh(c)  (bf16 directly)
                    nc.vector.scalar_tensor_tensor(
                        hw[:, half * 4:half * 4 + 4, :],
                        gv[:, :, 2, :], 1.0, tct[:],
                        mybir.AluOpType.add, mybir.AluOpType.mult,
                    )
                    nc.vector.tensor_scalar_mul(ch_half, c_half, 0.5)

                def phase(ph, t_idx):
                    hr, hw = hT[1 - ph], hT[ph]
                    # S1a part 1: lo chains, first half0 slots
                    wh_mms(aTlo[ph], hr, half_ms[0][:8], range(0, 4))
                    # S9: this step's score tail, kc 4..7 (needs hr[4..7])
                    sc_mms(sc2[ph], hr, range(4, 8))
                    # S1a part 2 + S3 part 1 (cover the softmax head)
                    wh_mms(aTlo[ph], hr, half_ms[0][8:], range(0, 4))
                    wh_mms(aThi[ph], hr, half_ms[0][:8], range(4, 8))
                    # softmax (DVE + Scalar), runs under S1a..S3.
                    # s_tot = 4*sum_n mask*(sc1+sc2); e_col = exp(s_tot)
                    junk = sp.tile([128, 2, NL], FP32, tag="junk")
                    s_tot = sp.tile([128, 1], FP32, tag="s_tot")
                    nc.vector.scalar_tensor_tensor(
                        junk[:], scz[ph][:, 0:2, :], 1.0, mask2_sb[:],
                        mybir.AluOpType.mult, mybir.AluOpType.mult,
                        accum_out=s_tot[:],
                    )
                    e_col = sp.tile([128, 1], FP32, tag="e_col")
                    nc.scalar.activation(
                        e_col[:], s_tot[:], mybir.ActivationFunctionType.Exp,
                    )
                    # S4: Z via block-diag partition sums
                    nc.tensor.matmul(
                        zsp[ph], bmask_sb[:], e_col[:], start=True, stop=True
                    )
                    # S3 part 2: hi chains, rest of half0
                    wh_mms(aThi[ph], hr, half_ms[0][8:], range(4, 8))
                    # gs(half0) partials fill the DVE while zsp is in flight
                    gs0 = gate_pre(ph, 0, t_idx)
                    rz = sp.tile([128, 1], FP32, tag="rz")
                    nc.vector.reciprocal(rz[:], zsp[ph])
                    ee = sp.tile([128, NL], BF16, tag="ee")
                    nc.vector.tensor_scalar(
                        ee[:], mask_sb[:], e_col[:, 0:1], rz[:, 0:1],
                        mybir.AluOpType.mult, mybir.AluOpType.mult,
                    )
                    # S5: attention matmuls half0 (early: starts gate chain)
                    e_mms(ph, ee, half_ms[0])
                    # V: finish gates half0 -> hw[0:4]
                    gate_fin(ph, 0, gs0, hw)
                    # S1b: lo chains, half1 (covers the half0 gate chain)
                    wh_mms(aTlo[ph], hr, half_ms[1], range(0, 4))
                    # S6: hi chains, half1
                    wh_mms(aThi[ph], hr, half_ms[1], range(4, 8))
                    # S7: attention matmuls half1
                    e_mms(ph, ee, half_ms[1])
                    gs1 = gate_pre(ph, 1, t_idx)
                    # S8: next step's scores kc 0..3 (consume hw half0)
                    sc_mms(sc1[1 - ph], hw, range(0, 4))
                    # V: finish gates half1 -> hw[4:8]
                    gate_fin(ph, 1, gs1, hw)
                    # output
                    nc.sync.dma_start(
                        out_d[bass.ds(t_idx, 1), :, :, :].rearrange(
                            "t p c n -> p (t c) n"
                        ),
                        hw[:],
                    )

                # prologue: scores kc 0..3 for step 0
                sc_mms(sc1[0], hT[1], range(0, 4))

                with tc.For_i(0, timesteps, 16, staggered_reset=True) as ti:
                    for k in range(16):
                        phase(k % 2, ti + k)

    nc.finalize()
    return nc


def prep_inputs(x, A, Wx, Wh, Wattn, b):
    """Host-side reshapes to device layouts; returns per-core input maps."""
    x = np.asarray(x, dtype=np.float32)
    A = np.asarray(A, dtype=np.float32)
    Wx = np.asarray(Wx, dtype=np.float32)
    Wh = np.asarray(Wh, dtype=np.float32)
    Wattn = np.asarray(Wattn, dtype=np.float32)
    b = np.asarray(b, dtype=np.float32)
    timesteps = x.shape[1]

    # weight layouts [p, kc, g] with k = kc*128 + p
    # per-gate-column scaling: i/f/o columns carry a 0.5 (tanh half-angle
    # trick), g columns stay full-scale; Wh gets an extra 0.5 (h2 = 2h).
    gsc = np.ones((G,), np.float32) * 0.5
    gsc[3 * H:] = 1.0
    wx_h = np.ascontiguousarray((gsc * Wx).reshape(DC, 128, G).transpose(1, 0, 2))
    wh_h = np.ascontiguousarray(
        ((0.5 * gsc) * Wh).reshape(HC, 128, G).transpose(1, 0, 2).astype(
            ml_dtypes.bfloat16
        )
    )
    wattn_h = np.ascontiguousarray(
        (gsc * Wattn).reshape(HC, 128, G).transpose(1, 0, 2)
    )
    b_h = np.ascontiguousarray((gsc * b).reshape(GM, 128).T)  # [p, m]
    mask_h = np.zeros((128, NL), dtype=np.float32)
    for p in range(128):
        mask_h[p, p // L] = 1.0
    bmask_h = (np.arange(128)[:, None] // L == np.arange(128)[None, :] // L).astype(
        np.float32
    )

    in_maps = []
    for c in range(NC):
        xs = x[c * NL:(c + 1) * NL]          # (8, T, 512)
        As = A[c * NL:(c + 1) * NL].reshape(NL, H, L)  # (8, 1024, 16)
        # xT [p, dc, n, t] = x[n, t, dc*128+p]
        xT_h = np.ascontiguousarray(
            xs.reshape(NL, timesteps, DC, 128).transpose(3, 2, 0, 1)
        )
        # afT [p, hc, n, l] = Af[n, hc*128+p, l]
        afT_h = np.ascontiguousarray(
            As.reshape(NL, HC, 128, L).transpose(2, 1, 0, 3)
        )
        in_maps.append(
            {
                "xT": xT_h,
                "afT": afT_h,
                "wx": wx_h,
                "wh": wh_h,
                "wattn": wattn_h,
                "bias": b_h,
                "mask": mask_h,
                "bmask": bmask_h,
            }
        )
    return in_maps


_NC_CACHE = {}


def kernel(x, A, Wx, Wh, Wattn, b, trace=False):
    timesteps = x.shape[1]
    key = timesteps
    if key not in _NC_CACHE:
        _NC_CACHE[key] = build_nc(timesteps)
    nc = _NC_CACHE[key]
    in_maps = prep_inputs(x, A, Wx, Wh, Wattn, b)
    res = run_bass_kernel_spmd(nc, in_maps, list(range(NC)), trace=trace)
    outs = []
    for c in range(NC):
        hsT = res.results[c]["hsT"]  # (T, 128, HC, NL)
        # out[n, t, hc*128+p] = hsT[t, p, hc, n]
        outs.append(0.5 * hsT.astype(np.float32).transpose(3, 0, 2, 1).reshape(NL, timesteps, H))
    full = np.concatenate(outs, axis=0).astype(np.float32)
    kernel.last_result = res
    return full
